# revision 5
# baseline (speedup 1.0000x reference)
"""Aitchison-Aitken categorical kernel on 8 TRN2 NeuronCores.

Math (reference, NUM_LEVELS=4, n_feat=64):
    w_f     = log(1-h_f) - log(h_f/3)                      (> 0 for h < 3/4)
    base    = sum_f log(h_f/3) - sum_f log(h_f) = -64*log(3)   (data independent!)
    match   = sum_f w_f * 1[test_if == train_jf]           ([n_test, n_train])
    ld      = match + base
    out     = rowmax(ld) * exp(ld - rowmax(ld))

Device algorithm (per core, data-parallel over test rows):
    - encode test/train as level-indicator matrices with K = 4*64 = 256
      contraction (2 K-tiles of 128: [lvl0|lvl1] and [lvl2|lvl3] stacked
      feature-major on partitions), fp16 (exact 0/1; w rounded to fp16)
    - match tile = 2-matmul PSUM accumulation (TensorE)
    - ebuf = exp(match + base)  (ScalarE, straight from PSUM, bf16 out;
      no row-max subtraction needed: ld is in [-150, 0]-ish, fp32/bf16 safe)
    - row max m = max(ebuf) via tensor_scalar accum_out (op1=max) at 4x DVE rate
    - c = ln(m)/m  (ScalarE Ln + DVE reciprocal); out = ebuf * c
      (identity: m=e^{maxld} => c*e^{ld} = maxld*e^{ld-maxld})

Sharding: test_Xs rows across 8 cores; bandwidths/train_Xs replicated;
out [1024, 8192] local per core, host-concatenated.
"""
import numpy as np
from contextlib import ExitStack

from concourse import bacc, mybir, masks, tile
from concourse.bass_utils import run_bass_kernel_spmd

f32 = mybir.dt.float32
f16 = mybir.dt.float16
bf16 = mybir.dt.bfloat16
ACTF = mybir.ActivationFunctionType
ALU = mybir.AluOpType

N_CORES = 8
N_TEST, N_TRAIN, N_FEAT = 8192, 8192, 64
M_LOC = N_TEST // N_CORES          # 1024 test rows per core
P = 128                            # partitions
M_TILES = M_LOC // P               # 8
NT = 512                           # train cols per matmul (one PSUM bank)
N_TILES = N_TRAIN // NT            # 16
QCOL = 2048                        # epilogue chunk (4 banks)
NQ = N_TRAIN // QCOL               # 4
BASE = float(-N_FEAT * np.log(3.0))

# fraction of the final multiply handed to GpSimd (cols, multiple of 512)
GP_COLS = 2048


def _build():
    nc = bacc.Bacc(None, target_bir_lowering=False)
    bw_ext = nc.declare_dram_parameter("bandwidths", [N_FEAT], f32, isOutput=False)
    test_ext = nc.declare_dram_parameter("test_Xs", [M_LOC, N_FEAT], f32, isOutput=False)
    train_ext = nc.declare_dram_parameter("train_Xs", [N_TRAIN, N_FEAT], f32, isOutput=False)
    out_ext = nc.declare_dram_parameter("out", [M_LOC, N_TRAIN], f32, isOutput=True)

    with tile.TileContext(nc) as tc, ExitStack() as ctx:
        const = ctx.enter_context(tc.tile_pool(name="const", bufs=1))
        enc = ctx.enter_context(tc.tile_pool(name="enc", bufs=1))
        stats = ctx.enter_context(tc.tile_pool(name="stats", bufs=4))
        ebuf_pool = ctx.enter_context(tc.tile_pool(name="ebuf", bufs=2))

        # ---- constants -------------------------------------------------
        ident = const.tile([P, P], f32)
        masks.make_identity(nc, ident[:])

        one_t = const.tile([N_FEAT, 1], f32)
        nc.vector.memset(one_t[:], 1.0)
        base_t = const.tile([P, 1], f32)
        nc.vector.memset(base_t[:], BASE)
        lvlA = const.tile([P, 1], f32)
        nc.vector.memset(lvlA[0:64, :], 0.0)
        nc.vector.memset(lvlA[64:128, :], 1.0)
        lvlB = const.tile([P, 1], f32)
        nc.vector.memset(lvlB[0:64, :], 2.0)
        nc.vector.memset(lvlB[64:128, :], 3.0)

        # ---- w vector from bandwidths ---------------------------------
        bw = const.tile([N_FEAT, 1], f32)
        nc.sync.dma_start(out=bw[:], in_=bw_ext[:].rearrange("(f o) -> f o", o=1))
        lt = const.tile([N_FEAT, 1], f32)   # log(1 - h)
        nc.scalar.activation(lt[:], bw[:], ACTF.Ln, bias=one_t[:], scale=-1.0)
        w2 = const.tile([P, 1], f32)
        lf = const.tile([N_FEAT, 1], f32)   # log(h/3)
        nc.scalar.activation(lf[:], bw[:], ACTF.Ln, scale=1.0 / 3.0)
        nc.vector.tensor_tensor(w2[0:64, :], lt[:], lf[:], op=ALU.subtract)
        nc.sync.dma_start(out=w2[64:128, :], in_=w2[0:64, :])

        # ---- encode: transpose + level indicators ----------------------
        # trainT/testT: [128, n] fp16 with the raw feature values duplicated
        # on both partition halves; Senc*/Tenc* are the indicator encodings.
        trainT = enc.tile([P, N_TRAIN], f16)
        sencA = enc.tile([P, N_TRAIN], f16)
        sencB = enc.tile([P, N_TRAIN], f16)
        testT = enc.tile([P, M_LOC], f16)
        tencA = enc.tile([P, M_LOC], f16)
        tencB = enc.tile([P, M_LOC], f16)

        with tc.tile_pool(name="prep", bufs=8) as prep, \
             tc.tile_pool(name="prep_ps", bufs=4, space="PSUM") as prep_ps:

            def transpose_encode(src_ext, n_rows, dstT, dstA, dstB, wmul):
                n_ch = n_rows // P              # 128-row chunks
                n_grp = n_ch // 4               # 4 chunks -> one [64, 512] psum
                for g in range(n_grp):
                    pt = prep_ps.tile([64, NT], f32)
                    for j in range(4):
                        c = g * 4 + j
                        chunk = prep.tile([P, N_FEAT], f32, tag="chunk")
                        nc.sync.dma_start(out=chunk[:], in_=src_ext[c * P:(c + 1) * P, :])
                        nc.tensor.transpose(pt[:, j * P:(j + 1) * P], chunk[:], ident[:])
                    # evacuate psum -> fp16 top half
                    nc.scalar.activation(dstT[0:64, g * NT:(g + 1) * NT], pt[:],
                                         ACTF.Copy, bias=0.0, scale=1.0)
                # duplicate to bottom half + encode, in 2048-col chunks
                n_q = (n_rows + QCOL - 1) // QCOL
                for q in range(n_q):
                    s = slice(q * QCOL, min((q + 1) * QCOL, n_rows))
                    nc.sync.dma_start(out=dstT[64:128, s], in_=dstT[0:64, s])
                    if wmul is None:
                        nc.vector.tensor_scalar(dstA[:, s], dstT[:, s], lvlA[:], None,
                                                op0=ALU.is_equal)
                        nc.vector.tensor_scalar(dstB[:, s], dstT[:, s], lvlB[:], None,
                                                op0=ALU.is_equal)
                    else:
                        nc.vector.tensor_scalar(dstA[:, s], dstT[:, s], lvlA[:], wmul[:],
                                                op0=ALU.is_equal, op1=ALU.mult)
                        nc.vector.tensor_scalar(dstB[:, s], dstT[:, s], lvlB[:], wmul[:],
                                                op0=ALU.is_equal, op1=ALU.mult)

            transpose_encode(test_ext, M_LOC, testT, tencA, tencB, w2)
            transpose_encode(train_ext, N_TRAIN, trainT, sencA, sencB, None)

        # ---- main loop --------------------------------------------------
        with tc.tile_pool(name="mm_ps", bufs=2, space="PSUM") as mm_ps:
            for m in range(M_TILES):
                ms = slice(m * P, (m + 1) * P)
                ebuf = ebuf_pool.tile([P, N_TRAIN], f32)
                pmax = stats.tile([P, NQ], f32)
                for q in range(NQ):
                    ps = mm_ps.tile([P, QCOL], f32)
                    for j in range(NQ):
                        n = q * 4 + j
                        ns = slice(n * NT, (n + 1) * NT)
                        js = slice(j * NT, (j + 1) * NT)
                        nc.tensor.matmul(ps[:, js], tencA[:, ms], sencA[:, ns],
                                         start=True, stop=False)
                        nc.tensor.matmul(ps[:, js], tencB[:, ms], sencB[:, ns],
                                         start=False, stop=True)
                    qs = slice(q * QCOL, (q + 1) * QCOL)
                    nc.scalar.activation(ebuf[:, qs], ps[:], ACTF.Exp,
                                         bias=base_t[:], scale=1.0)
                    nc.vector.tensor_scalar(ebuf[:, qs], ebuf[:, qs], 1.0, None,
                                            op0=ALU.mult, op1=ALU.max,
                                            accum_out=pmax[:, q:q + 1])
                # c = ln(max)/max
                mm_t = stats.tile([P, 1], f32)
                nc.vector.tensor_reduce(mm_t[:], pmax[:], axis=mybir.AxisListType.X,
                                        op=ALU.max)
                lnm = stats.tile([P, 1], f32)
                nc.scalar.activation(lnm[:], mm_t[:], ACTF.Ln)
                rec = stats.tile([P, 1], f32)
                nc.vector.reciprocal(rec[:], mm_t[:])
                cvec = stats.tile([P, 1], f32)
                nc.vector.tensor_tensor(cvec[:], lnm[:], rec[:], op=ALU.mult)

                dve_cols = N_TRAIN - GP_COLS
                for q in range(NQ):
                    lo, hi = q * QCOL, (q + 1) * QCOL
                    qs = slice(lo, hi)
                    if lo >= dve_cols:
                        nc.gpsimd.tensor_scalar(ebuf[:, qs], ebuf[:, qs], cvec[:],
                                                None, op0=ALU.mult)
                    else:
                        nc.vector.tensor_scalar(ebuf[:, qs], ebuf[:, qs], cvec[:],
                                                None, op0=ALU.mult)
                    nc.sync.dma_start(out=out_ext[ms, qs], in_=ebuf[:, qs])

    nc.compile()
    return nc


_NC = None


def _get_nc():
    global _NC
    if _NC is None:
        _NC = _build()
    return _NC


def kernel(bandwidths, test_Xs, train_Xs):
    bandwidths = np.ascontiguousarray(bandwidths, dtype=np.float32)
    test_Xs = np.ascontiguousarray(test_Xs, dtype=np.float32)
    train_Xs = np.ascontiguousarray(train_Xs, dtype=np.float32)

    nc = _get_nc()
    in_maps = [
        {
            "bandwidths": bandwidths,
            "test_Xs": np.ascontiguousarray(test_Xs[i * M_LOC:(i + 1) * M_LOC]),
            "train_Xs": train_Xs,
        }
        for i in range(N_CORES)
    ]
    res = run_bass_kernel_spmd(nc, in_maps, core_ids=list(range(N_CORES)))
    return np.concatenate([r["out"] for r in res.results], axis=0)


if __name__ == "__main__":
    rng = np.random.default_rng(0)
    h = rng.uniform(0.05, 0.5, N_FEAT).astype(np.float32)
    t = rng.integers(0, 4, (N_TEST, N_FEAT)).astype(np.float32)
    s = rng.integers(0, 4, (N_TRAIN, N_FEAT)).astype(np.float32)
    out = kernel(bandwidths=h, test_Xs=t, train_Xs=s)
    print(out.shape, out.dtype)


# revision 10
# speedup vs baseline: 2.1541x; 2.1541x over previous
"""Aitchison-Aitken categorical kernel on 8 TRN2 NeuronCores.

Math (reference, NUM_LEVELS=4, n_feat=64):
    w_f     = log(1-h_f) - log(h_f/3)                      (> 0 for h < 3/4)
    base    = sum_f log(h_f/3) - sum_f log(h_f) = -64*log(3)   (data independent!)
    match   = sum_f w_f * 1[test_if == train_jf]           ([n_test, n_train])
    ld      = match + base
    out     = rowmax(ld) * exp(ld - rowmax(ld))

Device algorithm (per core, data-parallel over test rows):
    - encode test/train as level-indicator matrices with K = 4*64 = 256
      contraction (2 K-tiles of 128: [lvl0|lvl1] and [lvl2|lvl3] stacked
      feature-major on partitions), fp16 (exact 0/1; w rounded to fp16)
    - match tile = 2-matmul PSUM accumulation (TensorE)
    - ebuf = exp(match + base)  (ScalarE, straight from PSUM, bf16 out;
      no row-max subtraction needed: ld is in [-150, 0]-ish, fp32/bf16 safe)
    - row max m = max(ebuf) via tensor_scalar accum_out (op1=max) at 4x DVE rate
    - c = ln(m)/m  (ScalarE Ln + DVE reciprocal); out = ebuf * c
      (identity: m=e^{maxld} => c*e^{ld} = maxld*e^{ld-maxld})

Sharding: test_Xs rows across 8 cores; bandwidths/train_Xs replicated;
out [1024, 8192] local per core, host-concatenated.
"""
import numpy as np
from contextlib import ExitStack

from concourse import bacc, mybir, masks, tile
from concourse.bass_utils import run_bass_kernel_spmd

f32 = mybir.dt.float32
f16 = mybir.dt.float16
bf16 = mybir.dt.bfloat16
ACTF = mybir.ActivationFunctionType
ALU = mybir.AluOpType

N_CORES = 8
N_TEST, N_TRAIN, N_FEAT = 8192, 8192, 64
M_LOC = N_TEST // N_CORES          # 1024 test rows per core
P = 128                            # partitions
M_TILES = M_LOC // P               # 8
NT = 512                           # train cols per matmul (one PSUM bank)
N_TILES = N_TRAIN // NT            # 16
QCOL = 2048                        # epilogue chunk (4 banks)
NQ = N_TRAIN // QCOL               # 4
BASE = float(-N_FEAT * np.log(3.0))

# fraction of the final multiply handed to GpSimd (cols, multiple of 512)
GP_COLS = 2048


def _build():
    nc = bacc.Bacc(None, target_bir_lowering=False)
    bw_ext = nc.declare_dram_parameter("bandwidths", [N_FEAT], f32, isOutput=False)
    test_ext = nc.declare_dram_parameter("test_Xs", [M_LOC, N_FEAT], f32, isOutput=False)
    train_ext = nc.declare_dram_parameter("train_Xs", [N_TRAIN, N_FEAT], f32, isOutput=False)
    out_ext = nc.declare_dram_parameter("out", [M_LOC, N_TRAIN], f32, isOutput=True)

    with tile.TileContext(nc) as tc, ExitStack() as ctx:
        const = ctx.enter_context(tc.tile_pool(name="const", bufs=1))
        enc = ctx.enter_context(tc.tile_pool(name="enc", bufs=1))
        stats = ctx.enter_context(tc.tile_pool(name="stats", bufs=4))
        ebuf_pool = ctx.enter_context(tc.tile_pool(name="ebuf", bufs=2))
        out_pool = ctx.enter_context(tc.tile_pool(name="obuf", bufs=2))

        # ---- constants -------------------------------------------------
        ident16 = const.tile([P, P], f16)
        masks.make_identity(nc, ident16[:])

        one_t = const.tile([N_FEAT, 1], f32)
        nc.vector.memset(one_t[:], 1.0)
        base_t = const.tile([P, 1], f32)
        nc.vector.memset(base_t[:], BASE)
        lvlA = const.tile([P, 1], f32)
        nc.vector.memset(lvlA[0:64, :], 0.0)
        nc.vector.memset(lvlA[64:128, :], 1.0)
        lvlB = const.tile([P, 1], f32)
        nc.vector.memset(lvlB[0:64, :], 2.0)
        nc.vector.memset(lvlB[64:128, :], 3.0)

        # ---- w vector from bandwidths ---------------------------------
        bw = const.tile([N_FEAT, 1], f32)
        nc.sync.dma_start(out=bw[:], in_=bw_ext[:].rearrange("(f o) -> f o", o=1))
        lt = const.tile([N_FEAT, 1], f32)   # log(1 - h)
        nc.scalar.activation(lt[:], bw[:], ACTF.Ln, bias=one_t[:], scale=-1.0)
        w2 = const.tile([P, 1], f32)
        lf = const.tile([N_FEAT, 1], f32)   # log(h/3)
        nc.scalar.activation(lf[:], bw[:], ACTF.Ln, scale=1.0 / 3.0)
        nc.vector.tensor_tensor(w2[0:64, :], lt[:], lf[:], op=ALU.subtract)
        nc.sync.dma_start(out=w2[64:128, :], in_=w2[0:64, :])

        # ---- encode: transpose + level indicators ----------------------
        # trainT/testT: [128, n] fp16 with the raw feature values duplicated
        # on both partition halves; Senc*/Tenc* are the indicator encodings.
        trainT = enc.tile([P, N_TRAIN], f16)
        sencA = enc.tile([P, N_TRAIN], f16)
        sencB = enc.tile([P, N_TRAIN], f16)
        testT = enc.tile([P, M_LOC], f16)
        tencA = enc.tile([P, M_LOC], f16)
        tencB = enc.tile([P, M_LOC], f16)

        with tc.tile_pool(name="prep", bufs=8) as prep, \
             tc.tile_pool(name="prep_ps", bufs=4, space="PSUM") as prep_ps:

            def transpose_encode(src_ext, n_rows, dstT, dstA, dstB, wmul):
                n_ch = n_rows // P              # 128-row chunks
                n_grp = n_ch // 4               # 4 chunks -> one [64, 512] psum
                for g in range(n_grp):
                    pt = prep_ps.tile([64, NT], f16)
                    for j in range(4):
                        c = g * 4 + j
                        chunk = prep.tile([P, N_FEAT], f32, tag="chunk")
                        nc.sync.dma_start(out=chunk[:], in_=src_ext[c * P:(c + 1) * P, :])
                        chunk16 = prep.tile([P, N_FEAT], f16, tag="chunk16")
                        nc.vector.tensor_copy(chunk16[:], chunk[:])
                        nc.tensor.transpose(pt[:, j * P:(j + 1) * P], chunk16[:],
                                            ident16[:])
                    # evacuate psum -> fp16 top half
                    nc.scalar.activation(dstT[0:64, g * NT:(g + 1) * NT], pt[:],
                                         ACTF.Copy, bias=0.0, scale=1.0)
                # duplicate to bottom half + encode, in 2048-col chunks
                n_q = (n_rows + QCOL - 1) // QCOL
                for q in range(n_q):
                    s = slice(q * QCOL, min((q + 1) * QCOL, n_rows))
                    nc.sync.dma_start(out=dstT[64:128, s], in_=dstT[0:64, s])
                    if wmul is None:
                        nc.vector.tensor_scalar(dstA[:, s], dstT[:, s], lvlA[:], None,
                                                op0=ALU.is_equal)
                        nc.vector.tensor_scalar(dstB[:, s], dstT[:, s], lvlB[:], None,
                                                op0=ALU.is_equal)
                    else:
                        nc.vector.tensor_scalar(dstA[:, s], dstT[:, s], lvlA[:], wmul[:],
                                                op0=ALU.is_equal, op1=ALU.mult)
                        nc.vector.tensor_scalar(dstB[:, s], dstT[:, s], lvlB[:], wmul[:],
                                                op0=ALU.is_equal, op1=ALU.mult)

            transpose_encode(test_ext, M_LOC, testT, tencA, tencB, w2)
            transpose_encode(train_ext, N_TRAIN, trainT, sencA, sencB, None)

        # ---- main loop --------------------------------------------------
        # PSUM: 4 tiles of [128, 1024] (2 banks each) pipelining
        # PE (4 matmuls) -> DVE (exact fp32 row-max partial) -> ACT (exp->bf16)
        PS_COL = 1024
        NPS = N_TRAIN // PS_COL            # 8 psum tiles per M-tile
        with tc.tile_pool(name="mm_ps", bufs=4, space="PSUM") as mm_ps:
            for m in range(M_TILES):
                ms = slice(m * P, (m + 1) * P)
                ebuf = ebuf_pool.tile([P, N_TRAIN], bf16)
                pmax = stats.tile([P, NPS], f32)
                for q in range(NPS):
                    ps = mm_ps.tile([P, PS_COL], f32)
                    for j in range(2):
                        n = q * 2 + j
                        ns = slice(n * NT, (n + 1) * NT)
                        js = slice(j * NT, (j + 1) * NT)
                        nc.tensor.matmul(ps[:, js], tencA[:, ms], sencA[:, ns],
                                         start=True, stop=False)
                        nc.tensor.matmul(ps[:, js], tencB[:, ms], sencB[:, ns],
                                         start=False, stop=True)
                    qs = slice(q * PS_COL, (q + 1) * PS_COL)
                    nc.vector.tensor_reduce(pmax[:, q:q + 1], ps[:],
                                            axis=mybir.AxisListType.X, op=ALU.max)
                    nc.scalar.activation(ebuf[:, qs], ps[:], ACTF.Exp,
                                         bias=base_t[:], scale=1.0)
                # c = maxld * exp(-maxld), maxld = max(match) + BASE (exact fp32)
                mm_t = stats.tile([P, 1], f32)
                nc.vector.tensor_reduce(mm_t[:], pmax[:], axis=mybir.AxisListType.X,
                                        op=ALU.max)
                mb_t = stats.tile([P, 1], f32)
                nc.vector.tensor_scalar(mb_t[:], mm_t[:], BASE, None, op0=ALU.add)
                em_t = stats.tile([P, 1], f32)
                nc.scalar.activation(em_t[:], mb_t[:], ACTF.Exp, scale=-1.0)
                cvec = stats.tile([P, 1], f32)
                nc.vector.tensor_tensor(cvec[:], mb_t[:], em_t[:], op=ALU.mult)

                obuf = out_pool.tile([P, N_TRAIN], f32)
                for q in range(NQ):
                    qs = slice(q * QCOL, (q + 1) * QCOL)
                    nc.vector.tensor_scalar(obuf[:, qs], ebuf[:, qs], cvec[:],
                                            None, op0=ALU.mult)
                    nc.sync.dma_start(out=out_ext[ms, qs], in_=obuf[:, qs])

    nc.compile()
    return nc


_NC = None


def _get_nc():
    global _NC
    if _NC is None:
        _NC = _build()
    return _NC


def kernel(bandwidths, test_Xs, train_Xs):
    bandwidths = np.ascontiguousarray(bandwidths, dtype=np.float32)
    test_Xs = np.ascontiguousarray(test_Xs, dtype=np.float32)
    train_Xs = np.ascontiguousarray(train_Xs, dtype=np.float32)

    nc = _get_nc()
    in_maps = [
        {
            "bandwidths": bandwidths,
            "test_Xs": np.ascontiguousarray(test_Xs[i * M_LOC:(i + 1) * M_LOC]),
            "train_Xs": train_Xs,
        }
        for i in range(N_CORES)
    ]
    res = run_bass_kernel_spmd(nc, in_maps, core_ids=list(range(N_CORES)))
    return np.concatenate([r["out"] for r in res.results], axis=0)


if __name__ == "__main__":
    rng = np.random.default_rng(0)
    h = rng.uniform(0.05, 0.5, N_FEAT).astype(np.float32)
    t = rng.integers(0, 4, (N_TEST, N_FEAT)).astype(np.float32)
    s = rng.integers(0, 4, (N_TRAIN, N_FEAT)).astype(np.float32)
    out = kernel(bandwidths=h, test_Xs=t, train_Xs=s)
    print(out.shape, out.dtype)


# revision 16
# speedup vs baseline: 2.6502x; 1.2303x over previous
"""Aitchison-Aitken categorical kernel on 8 TRN2 NeuronCores.

Math (reference, NUM_LEVELS=4, n_feat=64):
    w_f     = log(1-h_f) - log(h_f/3)                      (> 0 for h < 3/4)
    base    = sum_f log(h_f/3) - sum_f log(h_f) = -64*log(3)   (data independent!)
    match   = sum_f w_f * 1[test_if == train_jf]           ([n_test, n_train])
    ld      = match + base
    out     = rowmax(ld) * exp(ld - rowmax(ld))

Device algorithm (per core, data-parallel over test rows):
    - encode test/train as level-indicator matrices with K = 4*64 = 256
      contraction (2 K-tiles of 128: [lvl0|lvl1] and [lvl2|lvl3] stacked
      feature-major on partitions), fp16 (exact 0/1; w rounded to fp16)
    - match tile = 2-matmul PSUM accumulation (TensorE)
    - ebuf = exp(match + base)  (ScalarE, straight from PSUM, bf16 out;
      no row-max subtraction needed: ld is in [-150, 0]-ish, fp32/bf16 safe)
    - row max m = max(ebuf) via tensor_scalar accum_out (op1=max) at 4x DVE rate
    - c = ln(m)/m  (ScalarE Ln + DVE reciprocal); out = ebuf * c
      (identity: m=e^{maxld} => c*e^{ld} = maxld*e^{ld-maxld})

Sharding: test_Xs rows across 8 cores; bandwidths/train_Xs replicated;
out [1024, 8192] local per core, host-concatenated.
"""
import numpy as np
from contextlib import ExitStack

from concourse import bacc, mybir, masks, tile
from concourse.bass_utils import run_bass_kernel_spmd

f32 = mybir.dt.float32
f16 = mybir.dt.float16
bf16 = mybir.dt.bfloat16
ACTF = mybir.ActivationFunctionType
ALU = mybir.AluOpType

N_CORES = 8
N_TEST, N_TRAIN, N_FEAT = 8192, 8192, 64
M_LOC = N_TEST // N_CORES          # 1024 test rows per core
P = 128                            # partitions
M_TILES = M_LOC // P               # 8
NT = 512                           # train cols per matmul (one PSUM bank)
N_TILES = N_TRAIN // NT            # 16
QCOL = 2048                        # epilogue chunk (4 banks)
NQ = N_TRAIN // QCOL               # 4
BASE = float(-N_FEAT * np.log(3.0))

# fraction of the final multiply handed to GpSimd (cols, multiple of 512)
GP_COLS = 2048


def _build():
    nc = bacc.Bacc(None, target_bir_lowering=False)
    bw_ext = nc.declare_dram_parameter("bandwidths", [N_FEAT], f32, isOutput=False)
    test_ext = nc.declare_dram_parameter("test_Xs", [M_LOC, N_FEAT], f32, isOutput=False)
    train_ext = nc.declare_dram_parameter("train_Xs", [N_TRAIN, N_FEAT], f32, isOutput=False)
    out_ext = nc.declare_dram_parameter("out", [M_LOC, N_TRAIN], f32, isOutput=True)

    with tile.TileContext(nc) as tc, ExitStack() as ctx:
        const = ctx.enter_context(tc.tile_pool(name="const", bufs=1))
        enc = ctx.enter_context(tc.tile_pool(name="enc", bufs=1))
        stats = ctx.enter_context(tc.tile_pool(name="stats", bufs=4))
        ebuf_pool = ctx.enter_context(tc.tile_pool(name="ebuf", bufs=2))
        out_pool = ctx.enter_context(tc.tile_pool(name="obuf", bufs=2))

        # ---- constants -------------------------------------------------
        ident16 = const.tile([P, P], f16)
        masks.make_identity(nc, ident16[:])

        one_t = const.tile([N_FEAT, 1], f32)
        nc.vector.memset(one_t[:], 1.0)
        base_t = const.tile([P, 1], f32)
        nc.vector.memset(base_t[:], BASE)
        lvlA = const.tile([P, 1], f32)
        nc.vector.memset(lvlA[0:64, :], 0.0)
        nc.vector.memset(lvlA[64:128, :], 1.0)
        lvlB = const.tile([P, 1], f32)
        nc.vector.memset(lvlB[0:64, :], 2.0)
        nc.vector.memset(lvlB[64:128, :], 3.0)

        # ---- w vector from bandwidths ---------------------------------
        bw = const.tile([N_FEAT, 1], f32)
        nc.sync.dma_start(out=bw[:], in_=bw_ext[:].rearrange("(f o) -> f o", o=1))
        lt = const.tile([N_FEAT, 1], f32)   # log(1 - h)
        nc.scalar.activation(lt[:], bw[:], ACTF.Ln, bias=one_t[:], scale=-1.0)
        w2 = const.tile([P, 1], f32)
        lf = const.tile([N_FEAT, 1], f32)   # log(h/3)
        nc.scalar.activation(lf[:], bw[:], ACTF.Ln, scale=1.0 / 3.0)
        nc.vector.tensor_tensor(w2[0:64, :], lt[:], lf[:], op=ALU.subtract)
        nc.sync.dma_start(out=w2[64:128, :], in_=w2[0:64, :])

        # ---- encode: transpose + level indicators ----------------------
        # trainT/testT: [128, n] fp16 with the raw feature values duplicated
        # on both partition halves; Senc*/Tenc* are the indicator encodings.
        trainT = enc.tile([P, N_TRAIN], f16)
        sencA = enc.tile([P, N_TRAIN], f16)
        sencB = enc.tile([P, N_TRAIN], f16)
        testT = enc.tile([P, M_LOC], f16)
        tencA = enc.tile([P, M_LOC], f16)
        tencB = enc.tile([P, M_LOC], f16)

        with tc.tile_pool(name="prep", bufs=3) as prep, \
             tc.tile_pool(name="natp", bufs=1) as natp, \
             tc.tile_pool(name="prep_ps", bufs=4, space="PSUM") as prep_ps:

            def transpose_encode(src_ext, n_rows, dstT, dstA, dstB, wmul):
                n_ch = n_rows // P              # 128-row chunks
                # stage + cast natural-layout fp16 copy, 2048 rows at a time
                nat16 = natp.tile([P, n_ch * N_FEAT], f16,
                                  tag=f"nat16_{n_rows}")
                for g in range(max(1, n_ch // 16)):
                    rows = min(16, n_ch) * P
                    stage = prep.tile([P, rows * N_FEAT // P], f32, tag="stage")
                    nc.sync.dma_start(
                        out=stage[:].rearrange("p (c f) -> p c f", f=N_FEAT),
                        in_=src_ext[g * rows:(g + 1) * rows, :].rearrange(
                            "(c p) f -> p c f", p=P))
                    nc.vector.tensor_copy(
                        nat16[:, g * rows * N_FEAT // P:(g + 1) * rows * N_FEAT // P],
                        stage[:])
                n_grp = n_ch // 4               # 4 chunks -> one [64, 512] psum
                for g in range(n_grp):
                    pt = prep_ps.tile([64, NT], f16)
                    for j in range(4):
                        c = g * 4 + j
                        nc.tensor.transpose(pt[:, j * P:(j + 1) * P],
                                            nat16[:, c * N_FEAT:(c + 1) * N_FEAT],
                                            ident16[:])
                    # evacuate psum -> fp16 top half
                    nc.scalar.activation(dstT[0:64, g * NT:(g + 1) * NT], pt[:],
                                         ACTF.Copy, bias=0.0, scale=1.0)
                # duplicate to bottom half + encode, in 2048-col chunks
                n_q = (n_rows + QCOL - 1) // QCOL
                for q in range(n_q):
                    s = slice(q * QCOL, min((q + 1) * QCOL, n_rows))
                    nc.sync.dma_start(out=dstT[64:128, s], in_=dstT[0:64, s])
                    if wmul is None:
                        nc.vector.tensor_scalar(dstA[:, s], dstT[:, s], lvlA[:], None,
                                                op0=ALU.is_equal)
                        nc.vector.tensor_scalar(dstB[:, s], dstT[:, s], lvlB[:], None,
                                                op0=ALU.is_equal)
                    else:
                        nc.vector.tensor_scalar(dstA[:, s], dstT[:, s], lvlA[:], wmul[:],
                                                op0=ALU.is_equal, op1=ALU.mult)
                        nc.vector.tensor_scalar(dstB[:, s], dstT[:, s], lvlB[:], wmul[:],
                                                op0=ALU.is_equal, op1=ALU.mult)

            transpose_encode(test_ext, M_LOC, testT, tencA, tencB, w2)
            transpose_encode(train_ext, N_TRAIN, trainT, sencA, sencB, None)

        # ---- main loop --------------------------------------------------
        # PSUM: 4 tiles of [128, 1024] (2 banks each) pipelining
        # PE (4 matmuls) -> DVE (exact fp32 row-max partial) -> ACT (exp->bf16)
        PS_COL = 1024
        NPS = N_TRAIN // PS_COL            # 8 psum tiles per M-tile
        with tc.tile_pool(name="mm_ps", bufs=4, space="PSUM") as mm_ps:
            for m in range(M_TILES):
                ms = slice(m * P, (m + 1) * P)
                ebuf = ebuf_pool.tile([P, N_TRAIN], bf16)
                pmax = stats.tile([P, NPS], f32)
                for q in range(NPS):
                    ps = mm_ps.tile([P, PS_COL], f32)
                    # A,A,B,B order: reuse the stationary operand across banks
                    for j in range(2):
                        n = q * 2 + j
                        ns = slice(n * NT, (n + 1) * NT)
                        js = slice(j * NT, (j + 1) * NT)
                        nc.tensor.matmul(ps[:, js], tencA[:, ms], sencA[:, ns],
                                         start=True, stop=False)
                    for j in range(2):
                        n = q * 2 + j
                        ns = slice(n * NT, (n + 1) * NT)
                        js = slice(j * NT, (j + 1) * NT)
                        nc.tensor.matmul(ps[:, js], tencB[:, ms], sencB[:, ns],
                                         start=False, stop=True)
                    qs = slice(q * PS_COL, (q + 1) * PS_COL)
                    nc.vector.tensor_reduce(pmax[:, q:q + 1], ps[:],
                                            axis=mybir.AxisListType.X, op=ALU.max)
                    nc.scalar.activation(ebuf[:, qs], ps[:], ACTF.Exp,
                                         bias=base_t[:], scale=1.0)
                # c = maxld * exp(-maxld), maxld = max(match) + BASE (exact fp32)
                mm_t = stats.tile([P, 1], f32)
                nc.vector.tensor_reduce(mm_t[:], pmax[:], axis=mybir.AxisListType.X,
                                        op=ALU.max)
                mb_t = stats.tile([P, 1], f32)
                nc.vector.tensor_scalar(mb_t[:], mm_t[:], BASE, None, op0=ALU.add)
                em_t = stats.tile([P, 1], f32)
                nc.scalar.activation(em_t[:], mb_t[:], ACTF.Exp, scale=-1.0)
                cvec = stats.tile([P, 1], f32)
                nc.vector.tensor_tensor(cvec[:], mb_t[:], em_t[:], op=ALU.mult)

                obuf = out_pool.tile([P, N_TRAIN], f32)
                nc.vector.tensor_scalar(obuf[:], ebuf[:], cvec[:], None,
                                        op0=ALU.mult)
                for q in range(NQ):
                    qs = slice(q * QCOL, (q + 1) * QCOL)
                    nc.sync.dma_start(out=out_ext[ms, qs], in_=obuf[:, qs])

    nc.compile()
    return nc


_NC = None


def _get_nc():
    global _NC
    if _NC is None:
        _NC = _build()
    return _NC


def kernel(bandwidths, test_Xs, train_Xs):
    bandwidths = np.ascontiguousarray(bandwidths, dtype=np.float32)
    test_Xs = np.ascontiguousarray(test_Xs, dtype=np.float32)
    train_Xs = np.ascontiguousarray(train_Xs, dtype=np.float32)

    nc = _get_nc()
    in_maps = [
        {
            "bandwidths": bandwidths,
            "test_Xs": np.ascontiguousarray(test_Xs[i * M_LOC:(i + 1) * M_LOC]),
            "train_Xs": train_Xs,
        }
        for i in range(N_CORES)
    ]
    res = run_bass_kernel_spmd(nc, in_maps, core_ids=list(range(N_CORES)))
    return np.concatenate([r["out"] for r in res.results], axis=0)


if __name__ == "__main__":
    rng = np.random.default_rng(0)
    h = rng.uniform(0.05, 0.5, N_FEAT).astype(np.float32)
    t = rng.integers(0, 4, (N_TEST, N_FEAT)).astype(np.float32)
    s = rng.integers(0, 4, (N_TRAIN, N_FEAT)).astype(np.float32)
    out = kernel(bandwidths=h, test_Xs=t, train_Xs=s)
    print(out.shape, out.dtype)
